# revision 14
# baseline (speedup 1.0000x reference)
"""Correlation network kernel for Trainium2.

corr[b,i,j,k,l] = sum_c A[b,i,j,c] * B[b,k,l,c]

Per batch b this is  A_b (2304x64) @ B_b^T (64x2304) -> 2304x2304.
Sharding: data-parallel over batch B=8 across the 8 NeuronCores; each core
computes one full 2304x2304 correlation matrix, so the kernel is
output-write bound.

Device-side plan (per core):
  - Pure-bf16 compute with a bf16 DRAM output (upcast to fp32 on host).
    fro rel err ~3e-3 vs the fp32 reference (gate is 2e-2): bf16 input
    rounding ~2.4e-3 rms + bf16 output rounding ~1.1e-3 rms.  Halves the
    dominant HBM write (21.2 MB -> 10.6 MB/core) and cuts PE work 3x vs
    the previous hi/lo-split scheme.
  - Inputs arrive host-prepped in [C, HW] layout: lhsT packed [128, 1152]
    (rows 0:64 = even m-tiles, 64:128 = odd; K=C=64 so m-tiles pack in
    pairs into the 128-row PE array), rhs duplicated into both partition
    halves [128, 2304].  Loaded via the sync-engine HWDGE ring (the SP
    engine is otherwise idle until outputs start); the slow gpsimd SWDGE
    path (~670 ns/issue + late start) is avoided entirely.
  - Per (m-pair, 1024-col n-chunk): 4 bf16 matmuls (even rows 0:64 and
    odd rows 64:128, two 512-col PSUM banks each) into two 2-bank PSUM
    tiles, then one 1024-col PSUM fp32 -> SBUF bf16 copy per row
    (even rows on DVE, odd rows on ACT; Pool/gpsimd cannot read PSUM).
    2-bank copies amortize the ~250 ns per-instruction overhead.
  - Dual output streams: the sync HWDGE ring carries even rows (flushed
    {0:1024} after chunk 0 and {1024:2304} after chunk 2) plus odd-row
    tails; the scalar HWDGE ring carries one odd-row {0:2048} DMA per
    pair.  Two queues over the shared 16 DMA engines lift the sustained
    stream above the single-queue ~341 GB/s.  A short burst of dummy
    matmuls during the input-load window pre-warms the PE's HAM clock
    gate (cold PE runs at 1.2 GHz for the first ~3.4 us otherwise).
"""

import numpy as np
import ml_dtypes

import concourse.bacc as bacc
import concourse.mybir as mybir
import concourse.tile as tile
from concourse.bass_interp import get_hw_module
from concourse.bass_utils import run_bass_kernel_spmd

B, H, W, C = 8, 48, 48, 64
HW = H * W  # 2304
P = 128
M_TILES = HW // P  # 18
M_PAIRS = M_TILES // 2  # 9
N_TILE = 512
FP32 = mybir.dt.float32
BF16 = mybir.dt.bfloat16
BF16_NP = ml_dtypes.bfloat16

N_SPLITS = []
_n0 = 0
while _n0 < HW:
    N_SPLITS.append((_n0, min(N_TILE, HW - _n0)))
    _n0 += N_TILE


# n-chunks per row block: 2-bank PSUM tiles, one 1024-col bf16 matmul per
# row group per chunk (moving-operand max is 128x1024 for bf16).
N_CHUNKS = [(0, 1024), (1024, 1024), (2048, 256)]
# pair 0 splits its first chunk into 2x512 matmul/copy steps so the first
# output DMA can issue as soon as the first 512 input columns land.
N_CHUNKS_P0 = [(0, 512), (512, 512), (1024, 1024), (2048, 256)]


def _corr_body(tc, out, a_bf, b_bf):
    nc = tc.nc
    with (
        tc.tile_pool(name="ops", bufs=1) as op_pool,
        tc.tile_pool(name="ps", bufs=4, space="PSUM") as ps_pool,
        tc.tile_pool(name="outs", bufs=10) as out_pool,
    ):
        at = op_pool.tile([P, HW // 2], BF16)
        bt = op_pool.tile([P, HW], BF16)
        # Input loads split across both HWDGE rings so the first m-pair's
        # operands land as early as possible: sync carries what the first
        # matmuls need, scalar carries the rest in parallel.
        # Two DMAs per queue: every extra DMA boundary on the critical
        # path costs ~1.4 us of completion-semaphore latency.
        nc.sync.dma_start(out=at[:, 0:P], in_=a_bf[:, 0:P])
        nc.sync.dma_start(out=bt[:, 0 : 2 * N_TILE], in_=b_bf[:, 0 : 2 * N_TILE])
        nc.scalar.dma_start(out=at[:, P : HW // 2], in_=a_bf[:, P : HW // 2])
        nc.scalar.dma_start(out=bt[:, 2 * N_TILE : HW], in_=b_bf[:, 2 * N_TILE : HW])

        # PE prewarm: the HAM clock gate starts the PE at 1.2 GHz and only
        # ramps to 2.4 GHz after ~3.4 us of sustained activity.  Dummy
        # matmuls on a zeroed scratch tile during the otherwise-idle input
        # load window flip the gate before the real matmuls begin.
        warm = op_pool.tile([P, N_TILE], BF16)
        nc.gpsimd.memset(warm[:], 0.0)
        ps_w = ps_pool.tile([P, 2 * N_TILE], FP32, tag="ps")
        for w in range(4):
            nc.tensor.matmul(
                ps_w[:, :N_TILE],
                warm[0:64, 0:P],
                warm[0:64, :],
                start=True,
                stop=True,
            )

        for p in range(M_PAIRS):
            ot_e = out_pool.tile([P, HW], BF16, tag="ot")
            ot_o = out_pool.tile([P, HW], BF16, tag="ot")
            col = slice(p * P, (p + 1) * P)
            m_e, m_o = 2 * p, 2 * p + 1
            chunks = N_CHUNKS_P0 if p == 0 else N_CHUNKS
            for ni, (n0, nsz) in enumerate(chunks):
                ps_e = ps_pool.tile([P, 2 * N_TILE], FP32, tag="ps")
                ps_o = ps_pool.tile([P, 2 * N_TILE], FP32, tag="ps")
                for s0 in range(0, nsz, N_TILE):
                    ssz = min(N_TILE, nsz - s0)
                    nc.tensor.matmul(
                        ps_e[:, s0 : s0 + ssz],
                        at[0:64, col],
                        bt[0:64, n0 + s0 : n0 + s0 + ssz],
                        start=True,
                        stop=True,
                    )
                    nc.tensor.matmul(
                        ps_o[:, s0 : s0 + ssz],
                        at[64:128, col],
                        bt[64:128, n0 + s0 : n0 + s0 + ssz],
                        start=True,
                        stop=True,
                    )
                # even rows drain on DVE, odd rows on ACT: two independent
                # copy chains that run concurrently
                nc.vector.tensor_copy(ot_e[:, n0 : n0 + nsz], ps_e[:, :nsz])
                nc.scalar.copy(ot_o[:, n0 : n0 + nsz], ps_o[:, :nsz])

                # Dual output streams: sync's HWDGE ring carries even rows
                # (fine-grained flushes) plus odd-row tails; scalar's HWDGE
                # ring carries the bulk of odd rows (one o{0:2048} DMA per
                # pair, issued between ACT's copies).  Two queues over the
                # same 16 DMA engines lift the sustained stream above the
                # single-queue ~341 GB/s.  (gpsimd SWDGE is NOT used: it
                # drains at ~120 GB/s and its slow tail locks ot tiles.)
                if p == 0:
                    # First pair: stream each chunk to fill the ring
                    # while the pipeline ramps.
                    c0, c1 = n0, n0 + nsz
                    nc.sync.dma_start(
                        out=out[m_e * P : (m_e + 1) * P, c0:c1],
                        in_=ot_e[:, c0:c1],
                    )
                    nc.sync.dma_start(
                        out=out[m_o * P : (m_o + 1) * P, c0:c1],
                        in_=ot_o[:, c0:c1],
                    )
                elif ni == 0:
                    nc.sync.dma_start(
                        out=out[m_e * P : (m_e + 1) * P, 0:nsz],
                        in_=ot_e[:, 0:nsz],
                    )
                elif ni == 1:
                    nc.scalar.dma_start(
                        out=out[m_o * P : (m_o + 1) * P, 0 : n0 + nsz],
                        in_=ot_o[:, 0 : n0 + nsz],
                    )
                else:
                    nc.sync.dma_start(
                        out=out[m_e * P : (m_e + 1) * P, N_CHUNKS[0][1] : HW],
                        in_=ot_e[:, N_CHUNKS[0][1] : HW],
                    )
                    nc.sync.dma_start(
                        out=out[m_o * P : (m_o + 1) * P, n0:HW],
                        in_=ot_o[:, n0:HW],
                    )


_NC_CACHE = None


def _build():
    global _NC_CACHE
    if _NC_CACHE is None:
        nc = bacc.Bacc(
            "TRN2",
            target_bir_lowering=False,
            debug=False,
            enable_asserts=False,
        )
        a_bf = nc.dram_tensor("a_bf", [P, HW // 2], BF16, kind="ExternalInput").ap()
        b_bf = nc.dram_tensor("b_bf", [P, HW], BF16, kind="ExternalInput").ap()
        out = nc.dram_tensor("out", [HW, HW], BF16, kind="ExternalOutput").ap()
        with tile.TileContext(nc) as tc:
            _corr_body(tc, out, a_bf, b_bf)
        nc.compile()
        nc.m = get_hw_module(nc.m)
        _NC_CACHE = nc
    return _NC_CACHE


def _pack_lhs(xT):
    """[C, HW] -> [128, HW/2]: rows 0:64 even m-tiles, rows 64:128 odd."""
    t = xT.reshape(C, M_PAIRS, 2, P)  # [c, pair, eo, j]
    return np.ascontiguousarray(t.transpose(2, 0, 1, 3).reshape(2 * C, M_PAIRS * P))


def _pack_rhs(xT):
    """[C, HW] -> [128, HW]: duplicate into both partition halves."""
    return np.ascontiguousarray(np.concatenate([xT, xT], axis=0))


def _prep_inputs(feature_A, feature_B):
    in_maps = []
    for i in range(B):
        A2 = feature_A[i].reshape(HW, C).astype(BF16_NP)
        B2 = feature_B[i].reshape(HW, C).astype(BF16_NP)
        in_maps.append(
            {
                "a_bf": _pack_lhs(np.ascontiguousarray(A2.T)),
                "b_bf": _pack_rhs(np.ascontiguousarray(B2.T)),
            }
        )
    return in_maps


def _run(feature_A, feature_B, trace=False, **kwargs):
    feature_A = np.asarray(feature_A, dtype=np.float32)
    feature_B = np.asarray(feature_B, dtype=np.float32)
    assert feature_A.shape == (B, H, W, C), feature_A.shape
    assert feature_B.shape == (B, H, W, C), feature_B.shape

    nc = _build()
    in_maps = _prep_inputs(feature_A, feature_B)
    res = run_bass_kernel_spmd(nc, in_maps, list(range(B)), trace=trace, **kwargs)
    out = np.stack(
        [np.asarray(res.results[i]["out"]).astype(np.float32) for i in range(B)],
        axis=0,
    )
    return out.reshape(B, H, W, H, W), res


def kernel(feature_A, feature_B):
    out, _ = _run(feature_A, feature_B)
    return out


# revision 17
# speedup vs baseline: 1.1433x; 1.1433x over previous
"""Correlation network kernel for Trainium2.

corr[b,i,j,k,l] = sum_c A[b,i,j,c] * B[b,k,l,c]

Per batch b this is  A_b (2304x64) @ B_b^T (64x2304) -> 2304x2304.
Sharding: data-parallel over batch B=8 across the 8 NeuronCores; each core
computes one full 2304x2304 correlation matrix, so the kernel is
output-write bound.

Device-side plan (per core):
  - Pure-bf16 compute with a bf16 DRAM output (upcast to fp32 on host).
    fro rel err ~3e-3 vs the fp32 reference (gate is 2e-2): bf16 input
    rounding ~2.4e-3 rms + bf16 output rounding ~1.1e-3 rms.  Halves the
    dominant HBM write (21.2 MB -> 10.6 MB/core) and cuts PE work 3x vs
    the previous hi/lo-split scheme.
  - Inputs arrive host-prepped in [C, HW] layout: lhsT packed [128, 1152]
    (rows 0:64 = even m-tiles, 64:128 = odd; K=C=64 so m-tiles pack in
    pairs into the 128-row PE array), rhs duplicated into both partition
    halves [128, 2304].  Loaded via the sync-engine HWDGE ring (the SP
    engine is otherwise idle until outputs start); the slow gpsimd SWDGE
    path (~670 ns/issue + late start) is avoided entirely.
  - Per (m-pair, 1024-col n-chunk): 4 bf16 matmuls (even rows 0:64 and
    odd rows 64:128, two 512-col PSUM banks each) into two 2-bank PSUM
    tiles, then one 1024-col PSUM fp32 -> SBUF bf16 copy per row
    (even rows on DVE, odd rows on ACT; Pool/gpsimd cannot read PSUM).
    2-bank copies amortize the ~250 ns per-instruction overhead.
  - Output rides the sync HWDGE ring as a single stream (~341 GB/s
    sustained; a second concurrent stream interleaves the DRAM write
    pattern and lowers total bandwidth).  Each row flushes {0:1024} as
    soon as its first chunk is copied and {1024:2304} after its last,
    keeping the ring stocked; the first pair streams per-chunk and the
    last pair's odd rows drain on the scalar ring to share the tail.
    A short burst of dummy matmuls during the input-load window
    pre-warms the PE's HAM clock gate (cold PE runs at 1.2 GHz for the
    first ~3.4 us otherwise).
"""

import numpy as np
import ml_dtypes

import concourse.bacc as bacc
import concourse.mybir as mybir
import concourse.tile as tile
from concourse.bass_interp import get_hw_module
from concourse.bass_utils import run_bass_kernel_spmd

B, H, W, C = 8, 48, 48, 64
HW = H * W  # 2304
P = 128
M_TILES = HW // P  # 18
M_PAIRS = M_TILES // 2  # 9
N_TILE = 512
FP32 = mybir.dt.float32
BF16 = mybir.dt.bfloat16
BF16_NP = ml_dtypes.bfloat16

N_SPLITS = []
_n0 = 0
while _n0 < HW:
    N_SPLITS.append((_n0, min(N_TILE, HW - _n0)))
    _n0 += N_TILE


# n-chunks per row block: 2-bank PSUM tiles, one 1024-col bf16 matmul per
# row group per chunk (moving-operand max is 128x1024 for bf16).
N_CHUNKS = [(0, 1024), (1024, 1024), (2048, 256)]
# pair 0 splits its first chunk into 2x512 matmul/copy steps so the first
# output DMA can issue as soon as the first 512 input columns land.
N_CHUNKS_P0 = [(0, 512), (512, 512), (1024, 1024), (2048, 256)]


def _corr_body(tc, out, a_bf, b_bf):
    nc = tc.nc
    with (
        tc.tile_pool(name="ops", bufs=1) as op_pool,
        tc.tile_pool(name="ps", bufs=4, space="PSUM") as ps_pool,
        tc.tile_pool(name="outs", bufs=8) as out_pool,
    ):
        at = op_pool.tile([P, HW // 2], BF16)
        bt = op_pool.tile([P, HW], BF16)
        # Input loads split across both HWDGE rings so the first m-pair's
        # operands land as early as possible: sync carries what the first
        # matmuls need, scalar carries the rest in parallel.
        # Two DMAs per queue: every extra DMA boundary on the critical
        # path costs ~1.4 us of completion-semaphore latency.
        nc.sync.dma_start(out=at[:, 0:P], in_=a_bf[:, 0:P])
        nc.sync.dma_start(out=bt[:, 0 : 2 * N_TILE], in_=b_bf[:, 0 : 2 * N_TILE])
        nc.scalar.dma_start(out=at[:, P : HW // 2], in_=a_bf[:, P : HW // 2])
        nc.scalar.dma_start(out=bt[:, 2 * N_TILE : HW], in_=b_bf[:, 2 * N_TILE : HW])

        # PE prewarm: the HAM clock gate starts the PE at 1.2 GHz and only
        # ramps to 2.4 GHz after ~3.4 us of sustained activity.  Dummy
        # matmuls on a zeroed scratch tile during the otherwise-idle input
        # load window flip the gate before the real matmuls begin.
        warm = op_pool.tile([P, N_TILE], BF16)
        nc.gpsimd.memset(warm[:], 0.0)
        ps_w = ps_pool.tile([P, 2 * N_TILE], FP32, tag="ps")
        for w in range(4):
            nc.tensor.matmul(
                ps_w[:, :N_TILE],
                warm[0:64, 0:P],
                warm[0:64, :],
                start=True,
                stop=True,
            )

        for p in range(M_PAIRS):
            ot_e = out_pool.tile([P, HW], BF16, tag="ot")
            ot_o = out_pool.tile([P, HW], BF16, tag="ot")
            col = slice(p * P, (p + 1) * P)
            m_e, m_o = 2 * p, 2 * p + 1
            chunks = N_CHUNKS_P0 if p == 0 else N_CHUNKS
            for ni, (n0, nsz) in enumerate(chunks):
                ps_e = ps_pool.tile([P, 2 * N_TILE], FP32, tag="ps")
                ps_o = ps_pool.tile([P, 2 * N_TILE], FP32, tag="ps")
                for s0 in range(0, nsz, N_TILE):
                    ssz = min(N_TILE, nsz - s0)
                    nc.tensor.matmul(
                        ps_e[:, s0 : s0 + ssz],
                        at[0:64, col],
                        bt[0:64, n0 + s0 : n0 + s0 + ssz],
                        start=True,
                        stop=True,
                    )
                    nc.tensor.matmul(
                        ps_o[:, s0 : s0 + ssz],
                        at[64:128, col],
                        bt[64:128, n0 + s0 : n0 + s0 + ssz],
                        start=True,
                        stop=True,
                    )
                # even rows drain on DVE, odd rows on ACT: two independent
                # copy chains that run concurrently
                nc.vector.tensor_copy(ot_e[:, n0 : n0 + nsz], ps_e[:, :nsz])
                nc.scalar.copy(ot_o[:, n0 : n0 + nsz], ps_o[:, :nsz])

                # Single output stream on sync's HWDGE ring: a second
                # concurrent stream (scalar ring or gpsimd SWDGE) makes the
                # DRAM write pattern interleave across row blocks and DROPS
                # total bandwidth (measured 55 us vs 48.5 us single-queue).
                if p == 0:
                    # First pair: stream each chunk to fill the ring
                    # while the pipeline ramps.
                    flush = (n0, n0 + nsz)
                elif ni == 0:
                    # Flush the first 1024 cols as soon as chunk 0 lands;
                    # fine granularity keeps the DMA ring stocked so the
                    # 16 DMA engines never idle at transfer boundaries.
                    flush = (0, nsz)
                elif ni == len(chunks) - 1:
                    flush = (N_CHUNKS[0][1], HW)
                else:
                    flush = None
                if flush:
                    c0, c1 = flush
                    # last pair: odd rows drain on the scalar ring so the
                    # two HWDGE queues share only the final tail.
                    eng_o = nc.scalar if p == M_PAIRS - 1 else nc.sync
                    nc.sync.dma_start(
                        out=out[m_e * P : (m_e + 1) * P, c0:c1],
                        in_=ot_e[:, c0:c1],
                    )
                    eng_o.dma_start(
                        out=out[m_o * P : (m_o + 1) * P, c0:c1],
                        in_=ot_o[:, c0:c1],
                    )


_NC_CACHE = None


def _build():
    global _NC_CACHE
    if _NC_CACHE is None:
        nc = bacc.Bacc(
            "TRN2",
            target_bir_lowering=False,
            debug=False,
            enable_asserts=False,
        )
        a_bf = nc.dram_tensor("a_bf", [P, HW // 2], BF16, kind="ExternalInput").ap()
        b_bf = nc.dram_tensor("b_bf", [P, HW], BF16, kind="ExternalInput").ap()
        out = nc.dram_tensor("out", [HW, HW], BF16, kind="ExternalOutput").ap()
        with tile.TileContext(nc) as tc:
            _corr_body(tc, out, a_bf, b_bf)
        nc.compile()
        nc.m = get_hw_module(nc.m)
        _NC_CACHE = nc
    return _NC_CACHE


def _pack_lhs(xT):
    """[C, HW] -> [128, HW/2]: rows 0:64 even m-tiles, rows 64:128 odd."""
    t = xT.reshape(C, M_PAIRS, 2, P)  # [c, pair, eo, j]
    return np.ascontiguousarray(t.transpose(2, 0, 1, 3).reshape(2 * C, M_PAIRS * P))


def _pack_rhs(xT):
    """[C, HW] -> [128, HW]: duplicate into both partition halves."""
    return np.ascontiguousarray(np.concatenate([xT, xT], axis=0))


def _prep_inputs(feature_A, feature_B):
    in_maps = []
    for i in range(B):
        A2 = feature_A[i].reshape(HW, C).astype(BF16_NP)
        B2 = feature_B[i].reshape(HW, C).astype(BF16_NP)
        in_maps.append(
            {
                "a_bf": _pack_lhs(np.ascontiguousarray(A2.T)),
                "b_bf": _pack_rhs(np.ascontiguousarray(B2.T)),
            }
        )
    return in_maps


def _run(feature_A, feature_B, trace=False, **kwargs):
    feature_A = np.asarray(feature_A, dtype=np.float32)
    feature_B = np.asarray(feature_B, dtype=np.float32)
    assert feature_A.shape == (B, H, W, C), feature_A.shape
    assert feature_B.shape == (B, H, W, C), feature_B.shape

    nc = _build()
    in_maps = _prep_inputs(feature_A, feature_B)
    res = run_bass_kernel_spmd(nc, in_maps, list(range(B)), trace=trace, **kwargs)
    out = np.stack(
        [np.asarray(res.results[i]["out"]).astype(np.float32) for i in range(B)],
        axis=0,
    )
    return out.reshape(B, H, W, H, W), res


def kernel(feature_A, feature_B):
    out, _ = _run(feature_A, feature_B)
    return out


# revision 19
# speedup vs baseline: 1.1682x; 1.0218x over previous
"""Correlation network kernel for Trainium2.

corr[b,i,j,k,l] = sum_c A[b,i,j,c] * B[b,k,l,c]

Per batch b this is  A_b (2304x64) @ B_b^T (64x2304) -> 2304x2304.
Sharding: data-parallel over batch B=8 across the 8 NeuronCores; each core
computes one full 2304x2304 correlation matrix, so the kernel is
output-write bound.

Device-side plan (per core):
  - Pure-bf16 compute with a bf16 DRAM output (upcast to fp32 on host).
    fro rel err ~3e-3 vs the fp32 reference (gate is 2e-2): bf16 input
    rounding ~2.4e-3 rms + bf16 output rounding ~1.1e-3 rms.  Halves the
    dominant HBM write (21.2 MB -> 10.6 MB/core) and cuts PE work 3x vs
    the previous hi/lo-split scheme.
  - Inputs arrive host-prepped in [C, HW] layout: lhsT packed [128, 1152]
    (rows 0:64 = even m-tiles, 64:128 = odd; K=C=64 so m-tiles pack in
    pairs into the 128-row PE array), rhs duplicated into both partition
    halves [128, 2304].  Loaded via the two HWDGE rings (sync carries
    what the first matmuls need, scalar the rest in parallel); the slow
    gpsimd SWDGE path (~670 ns/issue + late start) is avoided entirely.
  - Per (m-pair, 1024-col n-chunk): 4 bf16 matmuls (even rows 0:64 and
    odd rows 64:128, two 512-col PSUM banks each) into two 2-bank PSUM
    tiles, then one 1024-col PSUM fp32 -> SBUF bf16 copy per row
    (even rows on DVE, odd rows on ACT; Pool/gpsimd cannot read PSUM).
    2-bank copies amortize the ~250 ns per-instruction overhead.
  - Output rides the sync HWDGE ring as a single stream (~341 GB/s
    sustained; a second concurrent stream interleaves the DRAM write
    pattern and lowers total bandwidth).  Each row flushes {0:1024} as
    soon as its first chunk is copied and {1024:2304} after its last,
    keeping the ring stocked; the first pair streams per-chunk and the
    last pair's odd rows drain on the scalar ring to share the tail.
    A short burst of dummy matmuls during the input-load window
    pre-warms the PE's HAM clock gate (cold PE runs at 1.2 GHz for the
    first ~3.4 us otherwise).
"""

import numpy as np
import ml_dtypes

import concourse.bacc as bacc
import concourse.mybir as mybir
import concourse.tile as tile
from concourse.bass_interp import get_hw_module
from concourse.bass_utils import run_bass_kernel_spmd

B, H, W, C = 8, 48, 48, 64
HW = H * W  # 2304
P = 128
M_TILES = HW // P  # 18
M_PAIRS = M_TILES // 2  # 9
N_TILE = 512
FP32 = mybir.dt.float32
BF16 = mybir.dt.bfloat16
BF16_NP = ml_dtypes.bfloat16

N_SPLITS = []
_n0 = 0
while _n0 < HW:
    N_SPLITS.append((_n0, min(N_TILE, HW - _n0)))
    _n0 += N_TILE


# n-chunks per row block: 2-bank PSUM tiles drained by one 1024-col copy
# each (matmuls stay at 512 cols: the ISA caps mm out elems per
# instruction at one PSUM bank).
N_CHUNKS = [(0, 1024), (1024, 1024), (2048, 256)]
# pair 0 splits its first chunk into 2x512 matmul/copy steps so the first
# output DMA can issue as soon as the first 512 input columns land.
N_CHUNKS_P0 = [(0, 512), (512, 512), (1024, 1024), (2048, 256)]


def _corr_body(tc, out, a_bf, b_bf):
    nc = tc.nc
    with (
        tc.tile_pool(name="ops", bufs=1) as op_pool,
        tc.tile_pool(name="ps", bufs=4, space="PSUM") as ps_pool,
        tc.tile_pool(name="outs", bufs=8) as out_pool,
    ):
        at = op_pool.tile([P, HW // 2], BF16)
        bt = op_pool.tile([P, HW], BF16)
        # Input loads split across both HWDGE rings so the first m-pair's
        # operands land as early as possible: sync carries what the first
        # matmuls need, scalar carries the rest in parallel.
        # Two DMAs per queue: every extra DMA boundary on the critical
        # path costs ~1.4 us of completion-semaphore latency.
        nc.sync.dma_start(out=at[:, 0:P], in_=a_bf[:, 0:P])
        nc.sync.dma_start(out=bt[:, 0 : 2 * N_TILE], in_=b_bf[:, 0 : 2 * N_TILE])
        nc.scalar.dma_start(out=at[:, P : HW // 2], in_=a_bf[:, P : HW // 2])
        nc.scalar.dma_start(out=bt[:, 2 * N_TILE : HW], in_=b_bf[:, 2 * N_TILE : HW])

        # PE prewarm: the HAM clock gate starts the PE at 1.2 GHz and only
        # ramps to 2.4 GHz after ~3.4 us of sustained activity.  Dummy
        # matmuls on a zeroed scratch tile during the otherwise-idle input
        # load window flip the gate before the real matmuls begin.
        warm = op_pool.tile([P, N_TILE], BF16)
        nc.gpsimd.memset(warm[:], 0.0)
        ps_w = ps_pool.tile([P, 2 * N_TILE], FP32, tag="ps")
        for w in range(4):
            nc.tensor.matmul(
                ps_w[:, :N_TILE],
                warm[0:64, 0:P],
                warm[0:64, :],
                start=True,
                stop=True,
            )

        for p in range(M_PAIRS):
            ot_e = out_pool.tile([P, HW], BF16, tag="ot")
            ot_o = out_pool.tile([P, HW], BF16, tag="ot")
            col = slice(p * P, (p + 1) * P)
            m_e, m_o = 2 * p, 2 * p + 1
            chunks = N_CHUNKS_P0 if p == 0 else N_CHUNKS
            for ni, (n0, nsz) in enumerate(chunks):
                ps_e = ps_pool.tile([P, 2 * N_TILE], FP32, tag="ps")
                ps_o = ps_pool.tile([P, 2 * N_TILE], FP32, tag="ps")
                for s0 in range(0, nsz, N_TILE):
                    ssz = min(N_TILE, nsz - s0)
                    nc.tensor.matmul(
                        ps_e[:, s0 : s0 + ssz],
                        at[0:64, col],
                        bt[0:64, n0 + s0 : n0 + s0 + ssz],
                        start=True,
                        stop=True,
                    )
                    nc.tensor.matmul(
                        ps_o[:, s0 : s0 + ssz],
                        at[64:128, col],
                        bt[64:128, n0 + s0 : n0 + s0 + ssz],
                        start=True,
                        stop=True,
                    )
                # even rows drain on DVE, odd rows on ACT: two independent
                # copy chains that run concurrently
                nc.vector.tensor_copy(ot_e[:, n0 : n0 + nsz], ps_e[:, :nsz])
                nc.scalar.copy(ot_o[:, n0 : n0 + nsz], ps_o[:, :nsz])

                # Single output stream on sync's HWDGE ring: a second
                # concurrent stream (scalar ring or gpsimd SWDGE) makes the
                # DRAM write pattern interleave across row blocks and DROPS
                # total bandwidth (measured 55 us vs 48.5 us single-queue).
                if p == 0:
                    # First pair: stream each chunk to fill the ring
                    # while the pipeline ramps.
                    flush = (n0, n0 + nsz)
                elif ni == 0:
                    # Flush the first 1024 cols as soon as chunk 0 lands;
                    # fine granularity keeps the DMA ring stocked so the
                    # 16 DMA engines never idle at transfer boundaries.
                    flush = (0, nsz)
                elif ni == len(chunks) - 1:
                    flush = (N_CHUNKS[0][1], HW)
                else:
                    flush = None
                if flush:
                    c0, c1 = flush
                    # last pair: odd rows drain on the scalar ring so the
                    # two HWDGE queues share only the final tail.
                    eng_o = nc.scalar if p == M_PAIRS - 1 else nc.sync
                    nc.sync.dma_start(
                        out=out[m_e * P : (m_e + 1) * P, c0:c1],
                        in_=ot_e[:, c0:c1],
                    )
                    eng_o.dma_start(
                        out=out[m_o * P : (m_o + 1) * P, c0:c1],
                        in_=ot_o[:, c0:c1],
                    )


_NC_CACHE = None


def _build():
    global _NC_CACHE
    if _NC_CACHE is None:
        nc = bacc.Bacc(
            "TRN2",
            target_bir_lowering=False,
            debug=False,
            enable_asserts=False,
        )
        a_bf = nc.dram_tensor("a_bf", [P, HW // 2], BF16, kind="ExternalInput").ap()
        b_bf = nc.dram_tensor("b_bf", [P, HW], BF16, kind="ExternalInput").ap()
        out = nc.dram_tensor("out", [HW, HW], BF16, kind="ExternalOutput").ap()
        with tile.TileContext(nc) as tc:
            _corr_body(tc, out, a_bf, b_bf)
        nc.compile()
        nc.m = get_hw_module(nc.m)
        _NC_CACHE = nc
    return _NC_CACHE


def _pack_lhs(xT):
    """[C, HW] -> [128, HW/2]: rows 0:64 even m-tiles, rows 64:128 odd."""
    t = xT.reshape(C, M_PAIRS, 2, P)  # [c, pair, eo, j]
    return np.ascontiguousarray(t.transpose(2, 0, 1, 3).reshape(2 * C, M_PAIRS * P))


def _pack_rhs(xT):
    """[C, HW] -> [128, HW]: duplicate into both partition halves."""
    return np.ascontiguousarray(np.concatenate([xT, xT], axis=0))


def _prep_inputs(feature_A, feature_B):
    in_maps = []
    for i in range(B):
        A2 = feature_A[i].reshape(HW, C).astype(BF16_NP)
        B2 = feature_B[i].reshape(HW, C).astype(BF16_NP)
        in_maps.append(
            {
                "a_bf": _pack_lhs(np.ascontiguousarray(A2.T)),
                "b_bf": _pack_rhs(np.ascontiguousarray(B2.T)),
            }
        )
    return in_maps


def _run(feature_A, feature_B, trace=False, **kwargs):
    feature_A = np.asarray(feature_A, dtype=np.float32)
    feature_B = np.asarray(feature_B, dtype=np.float32)
    assert feature_A.shape == (B, H, W, C), feature_A.shape
    assert feature_B.shape == (B, H, W, C), feature_B.shape

    nc = _build()
    in_maps = _prep_inputs(feature_A, feature_B)
    res = run_bass_kernel_spmd(nc, in_maps, list(range(B)), trace=trace, **kwargs)
    out = np.stack(
        [np.asarray(res.results[i]["out"]).astype(np.float32) for i in range(B)],
        axis=0,
    )
    return out.reshape(B, H, W, H, W), res


def kernel(feature_A, feature_B):
    out, _ = _run(feature_A, feature_B)
    return out
